# revision 10
# baseline (speedup 1.0000x reference)
"""Trainium2 Bass kernel for nn_MHA_28922309771622.

Multi-head attention with memory prefix (mems prepended to K/V), boolean
mask over KV positions, 16 heads, D=1024, S=2048, MEM=512, fp32.

Sharding: 8 cores = 2 (batch) x 4 (head blocks of 4 heads).  Each core
computes its head block's Q/K/V projections, attention, and two partial
transposed output projections woT[t].T @ ctx[t] -> [D, S]; the host sums
the 8 partials per batch, transposes, and adds bo.

Schedule (single PE stream, ACT-paced):
  - lead-in: k-outer KT(t=0) + Q(t=0) projections (few weight reloads,
    7 PSUM banks held), PE-warmup matmuls cover the input-DMA latency.
  - attention runs per (head-pair, s-half, kv-chunk).  Scores for the
    two heads of a pair are row-tiled (K=64 each, tile_position (0,0) /
    (64,0)) so both stream concurrently through the PE array.  Exp runs
    on the Scalar engine out of PSUM with the mask bias folded into the
    activation bias; the Scalar engine is the pacing engine mid-kernel
    (~92us of exp), so every remaining projection (V, KT t=1, Q t=1,
    out-proj t=0, patmul) is emitted as filler between score groups,
    sharing the scores' two PSUM slots.
  - V gets a ones-column (at a 128-col pitch) so the softmax denominator
    falls out of the context matmul's row 64 for free; denominators are
    reshaped [1,1024]->[128,8] via DMA straight out of PSUM so the
    reciprocal runs on all DVE lanes; a PE outer-product broadcasts the
    reciprocal back across partitions (patmul).
  - out-proj t=1 + remaining groups drain at the end, output DMA per
    128-row chunk overlaps the remaining matmuls.
"""

import contextlib
import sys

if "/opt/trn_rl_repo" not in sys.path:
    sys.path.insert(0, "/opt/trn_rl_repo")

import ml_dtypes
import numpy as np

import concourse.bass as bass  # noqa: F401
import concourse.mybir as mybir
import concourse.tile as tile
from concourse import bacc
from concourse.bass_utils import run_bass_kernel_spmd

B, S, MEM, D, H = 2, 2048, 512, 1024, 16
DH = D // H            # 64
SKV_FULL = MEM + S     # 2560
N_CORES = 8
HPC = 4                # heads per core
F = HPC * DH           # 256 features per core
NK = D // 128          # 8 contraction chunks over D
NT = F // 128          # 2 feature tiles (head pairs) of 128 per core
FP32 = mybir.dt.float32
BF16 = mybir.dt.bfloat16
NEG = -1.0e6


def _build(nj: int):
    """Build the SPMD Bass graph for skv_pad = nj*128 kv positions."""
    skv = nj * 128
    nc = bacc.Bacc("TRN2", target_bir_lowering=False, debug=False,
                   num_devices=N_CORES)

    def din(name, shape, dt=FP32):
        return nc.dram_tensor(name, list(shape), dt, kind="ExternalInput").ap()

    xT = din("xT", [D, S], BF16)      # x[b].T
    cT = din("cT", [D, skv], BF16)    # compacted concat(mems,x)[b].T
    wqT = din("wqT", [D, F], BF16)    # Wq[block].T
    wkT = din("wkT", [D, F], BF16)
    wvT = din("wvT", [D, F], BF16)
    woT = din("woT", [F, D], BF16)    # Wo[:, block].T
    mb = din("mb", [128, nj])         # exp bias: 0 kept, -1e6 padding
    ones64_d = din("ones64", [1, 64], BF16)
    bq2 = din("bq2", [128, NT])       # bq[block] feature-major [p, t]
    bk2 = din("bk2", [128, NT])
    bv2 = din("bv2", [128, NT])
    # two transposed partial outputs: out[t*D:(t+1)*D, :] = woT[t].T @ ctx[t]
    out = nc.dram_tensor("out", [NT * D, S], BF16, kind="ExternalOutput").ap()

    # kv-chunk N-splits for the KT projection (N<=512 per psum bank)
    kt_chunks = []
    off = 0
    while off < skv:
        ln = min(512, skv - off)
        kt_chunks.append((off, ln))
        off += ln
    NKC = len(kt_chunks)

    with tile.TileContext(nc) as tc, contextlib.ExitStack() as big:
        pers = big.enter_context(tc.tile_pool(name="pers", bufs=1))
        pql = big.enter_context(tc.tile_pool(name="pql", bufs=1))
        proj = big.enter_context(tc.tile_pool(name="proj", bufs=1))
        epool = big.enter_context(tc.tile_pool(name="epool", bufs=6))
        ostage = big.enter_context(tc.tile_pool(name="ostage", bufs=4))

        # ---------------- persistent tiles ----------------
        kt = [pers.tile([128, skv], BF16, name=f"kt{t}") for t in range(NT)]
        qtp = [pers.tile([128, S], BF16, name=f"qtp{t}") for t in range(NT)]
        # vaug layout per (j, h): [V_h (64) | ones | zeros (63)]
        vaug = pers.tile([128, nj * (HPC * 128)], BF16, name="vaug")
        wo_sb = [pers.tile([128, D], BF16, name=f"wo{t}") for t in range(NT)]
        ctxb = [pers.tile([128, S], BF16, name=f"ctxb{t}") for t in range(NT)]
        mb_sb = pers.tile([128, nj], FP32, name="mb_sb")
        bq_sb = pers.tile([128, NT], FP32, name="bq_sb")
        bk_sb = pers.tile([128, NT], FP32, name="bk_sb")
        bv_sb = pers.tile([128, NT], FP32, name="bv_sb")
        ones64_sb = pers.tile([1, 64], BF16, name="ones64_sb")
        # packed denominators: [128, h*16 + sh*8 + i] covers head h's
        # s-half sh as a [128, 8] block (reshaped via DMA from psum row 64)
        dpack = pers.tile([128, HPC * 16], FP32, name="dpack")
        rpack = pers.tile([128, HPC * 16], BF16, name="rpack")
        # recip[h] = 1/denominator for head h, [1, S] at partition 0 (the
        # patmul moving operand must share the weights' base partition)
        recip = [pers.tile([1, S], BF16, name=f"recip{h}")
                 for h in range(HPC)]
        dtmp = [pers.tile([1, S], FP32, name=f"dtmp{h}") for h in range(HPC)]
        scr = pers.tile([128, 2], FP32, name="scr")

        xt_sb = [pql.tile([128, S], BF16, name=f"xt{k}") for k in range(NK)]
        wq_sb = [pql.tile([128, F], BF16, name=f"wq{k}") for k in range(NK)]

        ct_sb = [proj.tile([128, skv], BF16, name=f"ct{k}")
                 for k in range(NK)]
        wv_sb = [proj.tile([128, F], BF16, name=f"wv{k}") for k in range(NK)]
        wk_sb = [proj.tile([128, F], BF16, name=f"wk{k}") for k in range(NK)]
        wz = proj.tile([128, 512], BF16, name="wz")

        vview = vaug.rearrange("p (j h e) -> p j h e", j=nj, h=HPC, e=128)

        # ============ warmup + DMAs ============
        with contextlib.ExitStack() as st1:
            lead = st1.enter_context(tc.tile_pool(name="lead", bufs=1,
                                                  space="PSUM"))

            # PE warmup: zero-dep matmuls keep the Tensor engine busy while
            # the input DMAs land (HAM clock gate).  wz memset must be the
            # first DVE op so the warmup isn't stuck behind the vaug memsets.
            nc.vector.memset(wz[:], 1.0)
            psw = lead.tile([128, 512], FP32, tag="psw", name="psw")
            for i in range(20):
                nc.tensor.matmul(psw[:], wz[:, :128], wz[:],
                                 start=(i == 0), stop=(i == 19))

            nc.vector.memset(vview[:, :, :, 64:65], 1.0)
            nc.vector.memset(vview[:, :, :, 65:128], 0.0)

            # small tiles first so nothing downstream waits on the bulk
            nc.sync.dma_start(mb_sb[:], mb[:])
            nc.sync.dma_start(bq_sb[:], bq2[:])
            nc.sync.dma_start(bk_sb[:], bk2[:])
            nc.sync.dma_start(bv_sb[:], bv2[:])
            nc.sync.dma_start(ones64_sb[:], ones64_d[:])
            for k in range(NK):
                nc.sync.dma_start(wk_sb[k][:], wkT[k * 128:(k + 1) * 128, :])
            for k in range(NK):
                nc.sync.dma_start(ct_sb[k][:], cT[k * 128:(k + 1) * 128, :])
            for k in range(NK):
                nc.sync.dma_start(wq_sb[k][:], wqT[k * 128:(k + 1) * 128, :])
            for half in range(4):
                hsl = slice(half * 512, (half + 1) * 512)
                for k in range(NK):
                    nc.sync.dma_start(xt_sb[k][:, hsl],
                                      xT[k * 128:(k + 1) * 128, hsl])
            for k in range(NK):
                nc.sync.dma_start(wv_sb[k][:], wvT[k * 128:(k + 1) * 128, :])
            for t in range(NT):
                nc.sync.dma_start(wo_sb[t][:], woT[t * 128:(t + 1) * 128, :])

            # preload the exp table set (~2.7us) during the lead-in
            nc.scalar.activation(scr[:, 1:2], mb_sb[:, 0:1],
                                 mybir.ActivationFunctionType.Exp)

            # ============ lead-in: KT t=0 + Q t=0, k-outer ============
            kps = [lead.tile([128, 512], FP32, tag=f"kps{c}", name=f"kps{c}")
                   for c in range(NKC)]
            qps = [lead.tile([128, 512], FP32, tag=f"qps{n}", name=f"qps{n}")
                   for n in range(4)]
            for k in range(NK):
                for c, (noff, nlen) in enumerate(kt_chunks):
                    nc.tensor.matmul(kps[c][:, :nlen], wk_sb[k][:, 0:128],
                                     ct_sb[k][:, noff:noff + nlen],
                                     start=(k == 0), stop=(k == NK - 1))
                for n in range(4):
                    nc.tensor.matmul(qps[n][:], wq_sb[k][:, 0:128],
                                     xt_sb[k][:, n * 512:(n + 1) * 512],
                                     start=(k == 0), stop=(k == NK - 1))
            for c, (noff, nlen) in enumerate(kt_chunks):
                nc.vector.tensor_scalar_add(
                    kt[0][:, noff:noff + nlen], kps[c][:, :nlen],
                    bk_sb[:, 0:1])
            for n in range(4):
                nc.vector.tensor_scalar_add(
                    qtp[0][:, n * 512:(n + 1) * 512], qps[n][:],
                    bq_sb[:, 0:1])

        # ============ attention (ACT-paced) with woven fillers ============
        # PSUM: ctx accumulators 2x[128,1024] (4 banks) + scores/filler
        # 2x[128,1024] (4 banks) = all 8 banks during attention.
        psctx = big.enter_context(tc.tile_pool(name="psctx", bufs=2,
                                               space="PSUM"))
        pssc = big.enter_context(tc.tile_pool(name="pssc", bufs=2,
                                              space="PSUM"))
        # filler emitters: each is a closure that emits one PE group using
        # one pssc slot; called between score groups.
        fillers = []

        def f_vproj(j):
            def emit():
                ps = pssc.tile([128, 256], FP32, tag="sc", name=f"vps{j}")
                for k in range(NK):
                    nc.tensor.matmul(
                        ps[:, :F], ct_sb[k][:, j * 128:(j + 1) * 128],
                        wv_sb[k][:], start=(k == 0), stop=(k == NK - 1))
                pv = ps[:, :F].rearrange("p (h e) -> p h e", h=HPC)
                nc.vector.tensor_copy(vview[:, j, :, 0:64], pv)
            return emit

        def f_ktproj(c):
            noff, nlen = kt_chunks[c]

            def emit():
                ps = pssc.tile([128, 512], FP32, tag="sc", name=f"k1ps{c}")
                for k in range(NK):
                    nc.tensor.matmul(ps[:, :nlen], wk_sb[k][:, 128:256],
                                     ct_sb[k][:, noff:noff + nlen],
                                     start=(k == 0), stop=(k == NK - 1))
                nc.vector.tensor_scalar_add(
                    kt[1][:, noff:noff + nlen], ps[:, :nlen], bk_sb[:, 1:2])
            return emit

        def f_qproj(n):
            def emit():
                ps = pssc.tile([128, 512], FP32, tag="sc", name=f"q1ps{n}")
                for k in range(NK):
                    nc.tensor.matmul(ps[:], wq_sb[k][:, 128:256],
                                     xt_sb[k][:, n * 512:(n + 1) * 512],
                                     start=(k == 0), stop=(k == NK - 1))
                nc.vector.tensor_scalar_add(
                    qtp[1][:, n * 512:(n + 1) * 512], ps[:], bq_sb[:, 1:2])
            return emit

        def f_patmul(t, h, sh):
            # broadcast 1/denominator across 64 partitions via PE outer
            # product, then normalize ctxb in place (two s-quarters).
            def emit():
                rsl = slice((h % 2) * 64, (h % 2) * 64 + 64)
                for q in range(2):
                    ssl = slice(sh * 1024 + q * 512, sh * 1024 + (q + 1) * 512)
                    ps = pssc.tile([128, 512], FP32, tag="sc",
                                   name=f"pat{h}{sh}{q}")
                    nc.tensor.matmul(ps[0:64, :], ones64_sb[:],
                                     recip[h][:, ssl],
                                     start=True, stop=True)
                    nc.vector.tensor_mul(ctxb[t][rsl, ssl],
                                         ctxb[t][rsl, ssl], ps[0:64, :])
            return emit

        def f_oproj(t, d):
            # transposed partial out-proj: out[t*D+d*128:..., :] rows
            def emit():
                dsl = slice(d * 128, (d + 1) * 128)
                for q in range(4):
                    ssl = slice(q * 512, (q + 1) * 512)
                    ps = pssc.tile([128, 512], FP32, tag="sc",
                                   name=f"ops{t}{d}{q}")
                    nc.tensor.matmul(ps[:], wo_sb[t][:, dsl],
                                     ctxb[t][:, ssl], start=True, stop=True)
                    ob = ostage.tile([128, 512], BF16, tag="ob",
                                     name=f"ob{t}{d}{q}")
                    if q % 2 == 0:
                        nc.vector.tensor_copy(ob[:], ps[:])
                    else:
                        nc.scalar.copy(ob[:], ps[:])
                    nc.sync.dma_start(out[t * D + d * 128:
                                          t * D + (d + 1) * 128, ssl], ob[:])
            return emit

        def take_filler():
            if fillers:
                fillers.pop(0)()

        # V chunk 0 must exist before the first ctx matmul
        f_vproj(0)()

        for t in range(NT):
            for sh in range(2):
                shsl = slice(sh * 1024, (sh + 1) * 1024)
                cps = [psctx.tile([128, 1024], FP32, tag="ctx",
                                  name=f"cps{t}{sh}{r}") for r in range(2)]
                etiles = [[None] * nj for _ in range(2)]
                for j in range(nj):
                    for r in range(2):
                        h = 2 * t + r
                        rsl = slice(r * 64, (r + 1) * 64)
                        ps = pssc.tile([128, 1024], FP32, tag="sc",
                                       name=f"sps{t}{sh}{j}{r}")
                        for q in range(2):
                            nc.tensor.matmul(
                                ps[:, q * 512:(q + 1) * 512],
                                kt[t][rsl, j * 128:(j + 1) * 128],
                                qtp[t][rsl, sh * 1024 + q * 512:
                                       sh * 1024 + (q + 1) * 512],
                                start=True, stop=True)
                        e = epool.tile([128, 1024], BF16, tag="expT",
                                       name=f"e{t}{sh}{j}{r}")
                        etiles[r][j] = e
                        nc.scalar.activation(
                            e[:], ps[:], mybir.ActivationFunctionType.Exp,
                            bias=mb_sb[:, j:j + 1], scale=1.0 / 8.0)
                    # one filler group between scores and ctx of each j
                    if t == 0 and sh == 0:
                        if j + 1 < nj:
                            f_vproj(j + 1)()
                    else:
                        take_filler()
                    for r in range(2):
                        h = 2 * t + r
                        vsl = vaug[:, j * (HPC * 128) + h * 128:
                                   j * (HPC * 128) + h * 128 + 65]
                        for q in range(2):
                            nc.tensor.matmul(
                                cps[r][0:65, q * 512:(q + 1) * 512],
                                vsl, etiles[r][j][:, q * 512:(q + 1) * 512],
                                start=(j == 0), stop=(j == nj - 1))
                # evict pair ctx + denominators for this s-half
                for r in range(2):
                    h = 2 * t + r
                    rsl = slice(r * 64, (r + 1) * 64)
                    nc.vector.tensor_copy(ctxb[t][rsl, shsl],
                                          cps[r][0:64, :])
                    # [1,1024] psum row -> sbuf, then -> [128,8] via DMA
                    # so the reciprocal runs on all DVE lanes
                    dsl = slice(h * 16 + sh * 8, h * 16 + sh * 8 + 8)
                    nc.vector.tensor_copy(dtmp[h][:, shsl],
                                          cps[r][64:65, :])
                    nc.sync.dma_start(dpack[:, dsl], dtmp[h][:, shsl])
                    with nc.allow_low_precision(
                            reason="bf16 recip feeds bf16 broadcast mm"):
                        nc.vector.reciprocal(rpack[:, dsl], dpack[:, dsl])
                    nc.sync.dma_start(recip[h][:, shsl], rpack[:, dsl])
                    fillers.insert(0, f_patmul(t, h, sh))
                if t == 0 and sh == 0:
                    # queue phase-2 projection fillers for pair 0's 2nd half
                    for c in range(NKC):
                        fillers.append(f_ktproj(c))
                    for n in range(4):
                        fillers.append(f_qproj(n))
            # pair done: flush any outstanding projection/patmul fillers so
            # pair t+1 has its K/Q and ctxb[t] is normalized
            while fillers:
                take_filler()
            nc.vector.tensor_scalar_add(ctxb[t][:], ctxb[t][:],
                                        bv_sb[:, t:t + 1])
            if t == 0:
                for d in range(NK):
                    fillers.append(f_oproj(0, d))
        while fillers:
            take_filler()
        for d in range(NK):
            f_oproj(1, d)()

    nc.compile()
    return nc


_CACHE = {}


def _graph(nj):
    if nj not in _CACHE:
        _CACHE[nj] = _build(nj)
    return _CACHE[nj]


def _prep_inputs(x, mems, mask, Wq, bq, Wk, bk, Wv, bv, Wo, bo):
    """Shard + preprocess on host. Returns (in_maps, nj)."""
    c = np.concatenate([mems, x], axis=1)          # [B, SKV_FULL, D]
    keep = [np.nonzero(mask[b] != 0)[0] for b in range(B)]
    n_eff = [len(k) for k in keep]
    nj = max(1, (max(n_eff) + 127) // 128)
    skv = nj * 128

    per_batch = []
    for b in range(B):
        ne = n_eff[b]
        cTb = np.zeros((D, skv), ml_dtypes.bfloat16)
        cTb[:, :ne] = c[b][keep[b]].T.astype(ml_dtypes.bfloat16)
        xTb = np.ascontiguousarray(x[b].T.astype(ml_dtypes.bfloat16))
        mbb = np.full(skv, NEG, np.float32)
        mbb[:ne] = 0.0
        mbb = np.ascontiguousarray(mbb.reshape(nj, 128).T)   # [128, nj]
        per_batch.append((xTb, cTb, mbb))

    def fmaj(v):   # [F] -> [128, NT] feature-major
        return np.ascontiguousarray(v.reshape(NT, 128).T.astype(np.float32))

    in_maps = []
    for core in range(N_CORES):
        b, hb = divmod(core, HPC)
        fs = slice(hb * F, (hb + 1) * F)
        xTb, cTb, mbb = per_batch[b]
        in_maps.append({
            "xT": xTb,
            "cT": cTb,
            "wqT": np.ascontiguousarray(Wq[fs, :].T.astype(ml_dtypes.bfloat16)),
            "wkT": np.ascontiguousarray(Wk[fs, :].T.astype(ml_dtypes.bfloat16)),
            "wvT": np.ascontiguousarray(Wv[fs, :].T.astype(ml_dtypes.bfloat16)),
            "woT": np.ascontiguousarray(Wo[:, fs].T.astype(ml_dtypes.bfloat16)),
            "mb": mbb,
            "ones64": np.ones((1, 64), ml_dtypes.bfloat16),
            "bq2": fmaj(bq[fs]),
            "bk2": fmaj(bk[fs]),
            "bv2": fmaj(bv[fs]),
        })
    return in_maps, nj


def _register_ntff_hook():
    try:
        from antenv.axon_hooks import (get_axon_ntff_profile_hook,
                                       set_axon_ntff_profile_hook)
    except ImportError:
        import types

        import antenv
        m = types.ModuleType("antenv.axon_hooks")
        m._hook = None
        m.set_axon_ntff_profile_hook = lambda h: setattr(m, "_hook", h)
        m.get_axon_ntff_profile_hook = lambda: m._hook
        sys.modules["antenv.axon_hooks"] = m
        antenv.axon_hooks = m
        get_axon_ntff_profile_hook = m.get_axon_ntff_profile_hook
        set_axon_ntff_profile_hook = m.set_axon_ntff_profile_hook
    if get_axon_ntff_profile_hook() is None:
        from trn_agent_boot.trn_boot import _ntff_profile_via_ctypes
        set_axon_ntff_profile_hook(
            _ntff_profile_via_ctypes("/opt/axon/libaxon_pjrt.so"))


def _run(inputs, trace=False, trace_kwargs=None):
    x = np.asarray(inputs["x"], np.float32)
    mems = np.asarray(inputs["mems"], np.float32)
    mask = np.asarray(inputs["mask"])
    Wq = np.asarray(inputs["Wq"], np.float32)
    bq = np.asarray(inputs["bq"], np.float32)
    Wk = np.asarray(inputs["Wk"], np.float32)
    bk = np.asarray(inputs["bk"], np.float32)
    Wv = np.asarray(inputs["Wv"], np.float32)
    bv = np.asarray(inputs["bv"], np.float32)
    Wo = np.asarray(inputs["Wo"], np.float32)
    bo = np.asarray(inputs["bo"], np.float32)

    in_maps, nj = _prep_inputs(x, mems, mask, Wq, bq, Wk, bk, Wv, bv, Wo, bo)
    nc = _graph(nj)

    if trace:
        _register_ntff_hook()

    res = run_bass_kernel_spmd(nc, in_maps, core_ids=list(range(N_CORES)),
                               trace=trace, **(trace_kwargs or {}))

    out = np.empty((B, S, D), np.float32)
    for b in range(B):
        acc = None
        for hb in range(HPC):
            o = res.results[b * HPC + hb]["out"].astype(np.float32)
            part = o[:D] + o[D:]
            acc = part if acc is None else acc + part
        out[b] = acc.T + bo[None, :]
    return out, res


def kernel(**inputs) -> np.ndarray:
    out, _ = _run(inputs, trace=False)
    return out


# revision 12
# speedup vs baseline: 1.0541x; 1.0541x over previous
"""Trainium2 Bass kernel for nn_MHA_28922309771622.

Multi-head attention with memory prefix (mems prepended to K/V), boolean
mask over KV positions, 16 heads, D=1024, S=2048, MEM=512, fp32.

Sharding: 8 cores = 2 (batch) x 4 (head blocks of 4 heads).  Each core
computes its head block's Q/K/V projections, attention, and two partial
transposed output projections woT[t].T @ ctx[t] -> [D, S]; the host sums
the 8 partials per batch, transposes, and adds bo.

Schedule (single PE stream, ACT-paced):
  - lead-in: k-outer KT(t=0) + Q(t=0) projections (few weight reloads,
    7 PSUM banks held), PE-warmup matmuls cover the input-DMA latency.
  - attention runs per (head-pair, s-half, kv-chunk).  Scores for the
    two heads of a pair are row-tiled (K=64 each, tile_position (0,0) /
    (64,0)) so both stream concurrently through the PE array.  Exp runs
    on the Scalar engine out of PSUM with the mask bias folded into the
    activation bias; the Scalar engine is the pacing engine mid-kernel
    (~92us of exp), so every remaining projection (V, KT t=1, Q t=1,
    out-proj t=0, patmul) is emitted as filler between score groups,
    sharing the scores' two PSUM slots.
  - V gets a ones-column (at a 128-col pitch) so the softmax denominator
    falls out of the context matmul's row 64 for free; denominators are
    reshaped [1,1024]->[128,8] via DMA straight out of PSUM so the
    reciprocal runs on all DVE lanes; a PE outer-product broadcasts the
    reciprocal back across partitions (patmul).
  - out-proj t=1 + remaining groups drain at the end, output DMA per
    128-row chunk overlaps the remaining matmuls.
"""

import contextlib
import sys

if "/opt/trn_rl_repo" not in sys.path:
    sys.path.insert(0, "/opt/trn_rl_repo")

import ml_dtypes
import numpy as np

import concourse.bass as bass  # noqa: F401
import concourse.mybir as mybir
import concourse.tile as tile
from concourse import bacc
from concourse.bass_utils import run_bass_kernel_spmd

B, S, MEM, D, H = 2, 2048, 512, 1024, 16
DH = D // H            # 64
SKV_FULL = MEM + S     # 2560
N_CORES = 8
HPC = 4                # heads per core
F = HPC * DH           # 256 features per core
NK = D // 128          # 8 contraction chunks over D
NT = F // 128          # 2 feature tiles (head pairs) of 128 per core
FP32 = mybir.dt.float32
BF16 = mybir.dt.bfloat16
NEG = -1.0e6


def _build(nj: int):
    """Build the SPMD Bass graph for skv_pad = nj*128 kv positions."""
    skv = nj * 128
    nc = bacc.Bacc("TRN2", target_bir_lowering=False, debug=False,
                   num_devices=N_CORES)

    def din(name, shape, dt=FP32):
        return nc.dram_tensor(name, list(shape), dt, kind="ExternalInput").ap()

    xT = din("xT", [D, S], BF16)      # x[b].T
    cT = din("cT", [D, skv], BF16)    # compacted concat(mems,x)[b].T
    wqT = din("wqT", [D, F], BF16)    # Wq[block].T
    wkT = din("wkT", [D, F], BF16)
    wvT = din("wvT", [D, F], BF16)
    woT = din("woT", [F, D], BF16)    # Wo[:, block].T
    mb = din("mb", [128, nj])         # exp bias: 0 kept, -1e6 padding
    ones64_d = din("ones64", [1, 64], BF16)
    bq2 = din("bq2", [128, NT])       # bq[block] feature-major [p, t]
    bk2 = din("bk2", [128, NT])
    bv2 = din("bv2", [128, NT])
    # two transposed partial outputs: out[t*D:(t+1)*D, :] = woT[t].T @ ctx[t]
    out = nc.dram_tensor("out", [NT * D, S], BF16, kind="ExternalOutput").ap()

    # kv-chunk N-splits for the KT projection (N<=512 per psum bank)
    kt_chunks = []
    off = 0
    while off < skv:
        ln = min(512, skv - off)
        kt_chunks.append((off, ln))
        off += ln
    NKC = len(kt_chunks)

    with tile.TileContext(nc) as tc, contextlib.ExitStack() as big:
        pers = big.enter_context(tc.tile_pool(name="pers", bufs=1))
        pql = big.enter_context(tc.tile_pool(name="pql", bufs=1))
        proj = big.enter_context(tc.tile_pool(name="proj", bufs=1))
        epool = big.enter_context(tc.tile_pool(name="epool", bufs=6))
        ostage = big.enter_context(tc.tile_pool(name="ostage", bufs=4))

        # ---------------- persistent tiles ----------------
        kt = [pers.tile([128, skv], BF16, name=f"kt{t}") for t in range(NT)]
        qtp = [pers.tile([128, S], BF16, name=f"qtp{t}") for t in range(NT)]
        # vaug layout per (j, h): [V_h (64) | ones | zeros (63)]
        vaug = pers.tile([128, nj * (HPC * 128)], BF16, name="vaug")
        wo_sb = [pers.tile([128, D], BF16, name=f"wo{t}") for t in range(NT)]
        ctxb = [pers.tile([128, S], BF16, name=f"ctxb{t}") for t in range(NT)]
        mb_sb = pers.tile([128, nj], FP32, name="mb_sb")
        bq_sb = pers.tile([128, NT], FP32, name="bq_sb")
        bk_sb = pers.tile([128, NT], FP32, name="bk_sb")
        bv_sb = pers.tile([128, NT], FP32, name="bv_sb")
        ones64_sb = pers.tile([1, 64], BF16, name="ones64_sb")
        # packed denominators: [128, h*16 + sh*8 + i] covers head h's
        # s-half sh as a [128, 8] block (reshaped via DMA from psum row 64)
        dpack = pers.tile([128, HPC * 16], FP32, name="dpack")
        rpack = pers.tile([128, HPC * 16], BF16, name="rpack")
        # recip[h] = 1/denominator for head h, [1, S] at partition 0 (the
        # patmul moving operand must share the weights' base partition)
        recip = [pers.tile([1, S], BF16, name=f"recip{h}")
                 for h in range(HPC)]
        dtmp = [pers.tile([1, S], FP32, name=f"dtmp{h}") for h in range(HPC)]
        scr = pers.tile([128, 2], FP32, name="scr")

        xt_sb = [pql.tile([128, S], BF16, name=f"xt{k}") for k in range(NK)]
        wq_sb = [pql.tile([128, F], BF16, name=f"wq{k}") for k in range(NK)]

        ct_sb = [proj.tile([128, skv], BF16, name=f"ct{k}")
                 for k in range(NK)]
        wv_sb = [proj.tile([128, F], BF16, name=f"wv{k}") for k in range(NK)]
        wk_sb = [proj.tile([128, F], BF16, name=f"wk{k}") for k in range(NK)]
        wz = proj.tile([128, 512], BF16, name="wz")

        vview = vaug.rearrange("p (j h e) -> p j h e", j=nj, h=HPC, e=128)

        # ============ warmup + DMAs ============
        with contextlib.ExitStack() as st1:
            lead = st1.enter_context(tc.tile_pool(name="lead", bufs=1,
                                                  space="PSUM"))

            # PE warmup: zero-dep matmuls keep the Tensor engine busy while
            # the input DMAs land (HAM clock gate).  wz memset must be the
            # first DVE op so the warmup isn't stuck behind the vaug memsets.
            nc.vector.memset(wz[:], 1.0)
            psw = lead.tile([128, 512], FP32, tag="psw", name="psw")
            for i in range(20):
                nc.tensor.matmul(psw[:], wz[:, :128], wz[:],
                                 start=(i == 0), stop=(i == 19))

            nc.vector.memset(vview[:, :, :, 64:65], 1.0)
            nc.vector.memset(vview[:, :, :, 65:128], 0.0)

            # small tiles first so nothing downstream waits on the bulk
            nc.sync.dma_start(mb_sb[:], mb[:])
            nc.sync.dma_start(bq_sb[:], bq2[:])
            nc.sync.dma_start(bk_sb[:], bk2[:])
            nc.sync.dma_start(bv_sb[:], bv2[:])
            nc.sync.dma_start(ones64_sb[:], ones64_d[:])
            for k in range(NK):
                nc.sync.dma_start(wk_sb[k][:], wkT[k * 128:(k + 1) * 128, :])
            for k in range(NK):
                nc.sync.dma_start(ct_sb[k][:], cT[k * 128:(k + 1) * 128, :])
            for k in range(NK):
                nc.sync.dma_start(wq_sb[k][:], wqT[k * 128:(k + 1) * 128, :])
            for half in range(4):
                hsl = slice(half * 512, (half + 1) * 512)
                for k in range(NK):
                    nc.sync.dma_start(xt_sb[k][:, hsl],
                                      xT[k * 128:(k + 1) * 128, hsl])
            for k in range(NK):
                nc.sync.dma_start(wv_sb[k][:], wvT[k * 128:(k + 1) * 128, :])
            for t in range(NT):
                nc.sync.dma_start(wo_sb[t][:], woT[t * 128:(t + 1) * 128, :])

            # preload the exp table set (~2.7us) during the lead-in
            nc.scalar.activation(scr[:, 1:2], mb_sb[:, 0:1],
                                 mybir.ActivationFunctionType.Exp)

            # ============ lead-in: KT t=0 + Q t=0, k-outer ============
            kps = [lead.tile([128, 512], FP32, tag=f"kps{c}", name=f"kps{c}")
                   for c in range(NKC)]
            qps = [lead.tile([128, 512], FP32, tag=f"qps{n}", name=f"qps{n}")
                   for n in range(4)]
            for k in range(NK):
                for c, (noff, nlen) in enumerate(kt_chunks):
                    nc.tensor.matmul(kps[c][:, :nlen], wk_sb[k][:, 0:128],
                                     ct_sb[k][:, noff:noff + nlen],
                                     start=(k == 0), stop=(k == NK - 1))
                for n in range(4):
                    nc.tensor.matmul(qps[n][:], wq_sb[k][:, 0:128],
                                     xt_sb[k][:, n * 512:(n + 1) * 512],
                                     start=(k == 0), stop=(k == NK - 1))
            for c, (noff, nlen) in enumerate(kt_chunks):
                nc.vector.tensor_scalar_add(
                    kt[0][:, noff:noff + nlen], kps[c][:, :nlen],
                    bk_sb[:, 0:1])
            for n in range(4):
                nc.vector.tensor_scalar_add(
                    qtp[0][:, n * 512:(n + 1) * 512], qps[n][:],
                    bq_sb[:, 0:1])

        # ============ attention (ACT-paced) with woven fillers ============
        # PSUM: ctx accumulators 2x[128,1024] (4 banks) + scores/filler
        # 2x[128,1024] (4 banks) = all 8 banks during attention.
        psctx = big.enter_context(tc.tile_pool(name="psctx", bufs=2,
                                               space="PSUM"))
        pssc = big.enter_context(tc.tile_pool(name="pssc", bufs=2,
                                              space="PSUM"))
        # filler emitters: (cost_us, closure) pairs; each closure emits one
        # PE group using one pssc slot; popped between score groups to keep
        # the PE array dense (HAM clock gate) while ACT paces on exp.
        fillers = []

        def f_vproj(j):
            def emit():
                ps = pssc.tile([128, 256], FP32, tag="sc", name=f"vps{j}")
                for k in range(NK):
                    nc.tensor.matmul(
                        ps[:, :F], ct_sb[k][:, j * 128:(j + 1) * 128],
                        wv_sb[k][:], start=(k == 0), stop=(k == NK - 1))
                pv = ps[:, :F].rearrange("p (h e) -> p h e", h=HPC)
                nc.vector.tensor_copy(vview[:, j, :, 0:64], pv)
            return emit

        def f_ktproj(c):
            noff, nlen = kt_chunks[c]

            def emit():
                ps = pssc.tile([128, 512], FP32, tag="sc", name=f"k1ps{c}")
                for k in range(NK):
                    nc.tensor.matmul(ps[:, :nlen], wk_sb[k][:, 128:256],
                                     ct_sb[k][:, noff:noff + nlen],
                                     start=(k == 0), stop=(k == NK - 1))
                nc.vector.tensor_scalar_add(
                    kt[1][:, noff:noff + nlen], ps[:, :nlen], bk_sb[:, 1:2])
            return emit

        def f_qproj(n):
            def emit():
                ps = pssc.tile([128, 512], FP32, tag="sc", name=f"q1ps{n}")
                for k in range(NK):
                    nc.tensor.matmul(ps[:], wq_sb[k][:, 128:256],
                                     xt_sb[k][:, n * 512:(n + 1) * 512],
                                     start=(k == 0), stop=(k == NK - 1))
                nc.vector.tensor_scalar_add(
                    qtp[1][:, n * 512:(n + 1) * 512], ps[:], bq_sb[:, 1:2])
            return emit

        def f_patmul(t, h, sh):
            # broadcast 1/denominator across 64 partitions via PE outer
            # product, then normalize ctxb in place (two s-quarters).
            def emit():
                rsl = slice((h % 2) * 64, (h % 2) * 64 + 64)
                for q in range(2):
                    ssl = slice(sh * 1024 + q * 512, sh * 1024 + (q + 1) * 512)
                    ps = pssc.tile([128, 512], FP32, tag="sc",
                                   name=f"pat{h}{sh}{q}")
                    nc.tensor.matmul(ps[0:64, :], ones64_sb[:],
                                     recip[h][:, ssl],
                                     start=True, stop=True)
                    nc.vector.tensor_mul(ctxb[t][rsl, ssl],
                                         ctxb[t][rsl, ssl], ps[0:64, :])
            return emit

        def f_oproj(t, d, tail=False):
            # transposed partial out-proj: out[t*D+d*128:..., :] rows
            def emit():
                dsl = slice(d * 128, (d + 1) * 128)
                for q in range(4):
                    ssl = slice(q * 512, (q + 1) * 512)
                    ps = pssc.tile([128, 512], FP32, tag="sc",
                                   name=f"ops{t}{d}{q}")
                    nc.tensor.matmul(ps[:], wo_sb[t][:, dsl],
                                     ctxb[t][:, ssl], start=True, stop=True)
                    ob = ostage.tile([128, 512], BF16, tag="ob",
                                     name=f"ob{t}{d}{q}")
                    if tail and q % 2 == 1:
                        # ACT is idle in the tail; mid-attention it paces exp
                        nc.scalar.copy(ob[:], ps[:])
                    else:
                        nc.vector.tensor_copy(ob[:], ps[:])
                    nc.sync.dma_start(out[t * D + d * 128:
                                          t * D + (d + 1) * 128, ssl], ob[:])
            return emit

        def f_dummy():
            # PE keep-warm: two matmuls into a dead psum slot (wz is ones)
            ps = pssc.tile([128, 512], FP32, tag="sc", name="dps")
            for i in range(2):
                nc.tensor.matmul(ps[:], wz[:, :128], wz[:],
                                 start=(i == 0), stop=(i == 1))

        def take_fillers(budget):
            got = 0.0
            while fillers and got < budget:
                cost, emit = fillers.pop(0)
                emit()
                got += cost
            if got == 0.0 and budget > 0.8:
                f_dummy()

        # V chunk 0 must exist before the first ctx matmul
        f_vproj(0)()

        for t in range(NT):
            for sh in range(2):
                shsl = slice(sh * 1024, (sh + 1) * 1024)
                cps = [psctx.tile([128, 1024], FP32, tag="ctx",
                                  name=f"cps{t}{sh}{r}") for r in range(2)]
                etiles = [[None] * nj for _ in range(2)]

                def emit_ctx(j, t=t, cps=cps, etiles=etiles):
                    for r in range(2):
                        h = 2 * t + r
                        vsl = vaug[:, j * (HPC * 128) + h * 128:
                                   j * (HPC * 128) + h * 128 + 65]
                        for q in range(2):
                            nc.tensor.matmul(
                                cps[r][0:65, q * 512:(q + 1) * 512],
                                vsl, etiles[r][j][:, q * 512:(q + 1) * 512],
                                start=(j == 0), stop=(j == nj - 1))

                for j in range(nj):
                    for r in range(2):
                        rsl = slice(r * 64, (r + 1) * 64)
                        ps = pssc.tile([128, 1024], FP32, tag="sc",
                                       name=f"sps{t}{sh}{j}{r}")
                        for q in range(2):
                            nc.tensor.matmul(
                                ps[:, q * 512:(q + 1) * 512],
                                kt[t][rsl, j * 128:(j + 1) * 128],
                                qtp[t][rsl, sh * 1024 + q * 512:
                                       sh * 1024 + (q + 1) * 512],
                                start=True, stop=True)
                        e = epool.tile([128, 1024], BF16, tag="expT",
                                       name=f"e{t}{sh}{j}{r}")
                        etiles[r][j] = e
                        nc.scalar.activation(
                            e[:], ps[:], mybir.ActivationFunctionType.Exp,
                            bias=mb_sb[:, j:j + 1], scale=1.0 / 8.0)
                    # ctx for the previous chunk: its exp finished an
                    # iteration ago, so the PE never stalls on ACT here
                    if j > 0:
                        emit_ctx(j - 1)
                    # top up the PE with projection/out-proj filler work
                    if t == 0 and sh == 0:
                        if j + 1 < nj:
                            f_vproj(j + 1)()
                        else:
                            take_fillers(0.9)
                    else:
                        take_fillers(0.9 if j > 0 else 0.4)
                emit_ctx(nj - 1)
                # evict pair ctx + denominators for this s-half
                for r in range(2):
                    h = 2 * t + r
                    rsl = slice(r * 64, (r + 1) * 64)
                    nc.vector.tensor_copy(ctxb[t][rsl, shsl],
                                          cps[r][0:64, :])
                    # [1,1024] psum row -> sbuf, then -> [128,8] via DMA
                    # so the reciprocal runs on all DVE lanes
                    dsl = slice(h * 16 + sh * 8, h * 16 + sh * 8 + 8)
                    nc.vector.tensor_copy(dtmp[h][:, shsl],
                                          cps[r][64:65, :])
                    nc.sync.dma_start(dpack[:, dsl], dtmp[h][:, shsl])
                    with nc.allow_low_precision(
                            reason="bf16 recip feeds bf16 broadcast mm"):
                        nc.vector.reciprocal(rpack[:, dsl], dpack[:, dsl])
                    nc.sync.dma_start(recip[h][:, shsl], rpack[:, dsl])
                    fillers.insert(0, (0.5, f_patmul(t, h, sh)))
                if t == 0 and sh == 0:
                    # queue phase-2 projection fillers for pair 0's 2nd half
                    for c in range(NKC):
                        fillers.append((1.5, f_ktproj(c)))
                    for n in range(4):
                        fillers.append((1.8, f_qproj(n)))
            # pair done: flush any outstanding projection/patmul fillers so
            # pair t+1 has its K/Q and ctxb[t] is normalized
            while fillers:
                fillers.pop(0)[1]()
            nc.vector.tensor_scalar_add(ctxb[t][:], ctxb[t][:],
                                        bv_sb[:, t:t + 1])
            if t == 0:
                for d in range(NK):
                    fillers.append((1.8, f_oproj(0, d)))
        while fillers:
            fillers.pop(0)[1]()
        for d in range(NK):
            f_oproj(1, d, tail=True)()

    nc.compile()
    return nc


_CACHE = {}


def _graph(nj):
    if nj not in _CACHE:
        _CACHE[nj] = _build(nj)
    return _CACHE[nj]


def _prep_inputs(x, mems, mask, Wq, bq, Wk, bk, Wv, bv, Wo, bo):
    """Shard + preprocess on host. Returns (in_maps, nj)."""
    c = np.concatenate([mems, x], axis=1)          # [B, SKV_FULL, D]
    keep = [np.nonzero(mask[b] != 0)[0] for b in range(B)]
    n_eff = [len(k) for k in keep]
    nj = max(1, (max(n_eff) + 127) // 128)
    skv = nj * 128

    per_batch = []
    for b in range(B):
        ne = n_eff[b]
        cTb = np.zeros((D, skv), ml_dtypes.bfloat16)
        cTb[:, :ne] = c[b][keep[b]].T.astype(ml_dtypes.bfloat16)
        xTb = np.ascontiguousarray(x[b].T.astype(ml_dtypes.bfloat16))
        mbb = np.full(skv, NEG, np.float32)
        mbb[:ne] = 0.0
        mbb = np.ascontiguousarray(mbb.reshape(nj, 128).T)   # [128, nj]
        per_batch.append((xTb, cTb, mbb))

    def fmaj(v):   # [F] -> [128, NT] feature-major
        return np.ascontiguousarray(v.reshape(NT, 128).T.astype(np.float32))

    in_maps = []
    for core in range(N_CORES):
        b, hb = divmod(core, HPC)
        fs = slice(hb * F, (hb + 1) * F)
        xTb, cTb, mbb = per_batch[b]
        in_maps.append({
            "xT": xTb,
            "cT": cTb,
            "wqT": np.ascontiguousarray(Wq[fs, :].T.astype(ml_dtypes.bfloat16)),
            "wkT": np.ascontiguousarray(Wk[fs, :].T.astype(ml_dtypes.bfloat16)),
            "wvT": np.ascontiguousarray(Wv[fs, :].T.astype(ml_dtypes.bfloat16)),
            "woT": np.ascontiguousarray(Wo[:, fs].T.astype(ml_dtypes.bfloat16)),
            "mb": mbb,
            "ones64": np.ones((1, 64), ml_dtypes.bfloat16),
            "bq2": fmaj(bq[fs]),
            "bk2": fmaj(bk[fs]),
            "bv2": fmaj(bv[fs]),
        })
    return in_maps, nj


def _register_ntff_hook():
    try:
        from antenv.axon_hooks import (get_axon_ntff_profile_hook,
                                       set_axon_ntff_profile_hook)
    except ImportError:
        import types

        import antenv
        m = types.ModuleType("antenv.axon_hooks")
        m._hook = None
        m.set_axon_ntff_profile_hook = lambda h: setattr(m, "_hook", h)
        m.get_axon_ntff_profile_hook = lambda: m._hook
        sys.modules["antenv.axon_hooks"] = m
        antenv.axon_hooks = m
        get_axon_ntff_profile_hook = m.get_axon_ntff_profile_hook
        set_axon_ntff_profile_hook = m.set_axon_ntff_profile_hook
    if get_axon_ntff_profile_hook() is None:
        from trn_agent_boot.trn_boot import _ntff_profile_via_ctypes
        set_axon_ntff_profile_hook(
            _ntff_profile_via_ctypes("/opt/axon/libaxon_pjrt.so"))


def _run(inputs, trace=False, trace_kwargs=None):
    x = np.asarray(inputs["x"], np.float32)
    mems = np.asarray(inputs["mems"], np.float32)
    mask = np.asarray(inputs["mask"])
    Wq = np.asarray(inputs["Wq"], np.float32)
    bq = np.asarray(inputs["bq"], np.float32)
    Wk = np.asarray(inputs["Wk"], np.float32)
    bk = np.asarray(inputs["bk"], np.float32)
    Wv = np.asarray(inputs["Wv"], np.float32)
    bv = np.asarray(inputs["bv"], np.float32)
    Wo = np.asarray(inputs["Wo"], np.float32)
    bo = np.asarray(inputs["bo"], np.float32)

    in_maps, nj = _prep_inputs(x, mems, mask, Wq, bq, Wk, bk, Wv, bv, Wo, bo)
    nc = _graph(nj)

    if trace:
        _register_ntff_hook()

    res = run_bass_kernel_spmd(nc, in_maps, core_ids=list(range(N_CORES)),
                               trace=trace, **(trace_kwargs or {}))

    out = np.empty((B, S, D), np.float32)
    for b in range(B):
        acc = None
        for hb in range(HPC):
            o = res.results[b * HPC + hb]["out"].astype(np.float32)
            part = o[:D] + o[D:]
            acc = part if acc is None else acc + part
        out[b] = acc.T + bo[None, :]
    return out, res


def kernel(**inputs) -> np.ndarray:
    out, _ = _run(inputs, trace=False)
    return out


# revision 15
# speedup vs baseline: 1.1487x; 1.0898x over previous
"""Trainium2 Bass kernel for nn_MHA_28922309771622.

Multi-head attention with memory prefix (mems prepended to K/V), boolean
mask over KV positions, 16 heads, D=1024, S=2048, MEM=512, fp32.

Sharding: 8 cores = 2 (batch) x 4 (head blocks of 4 heads).  Each core
computes its head block's Q/K/V projections, attention, and two partial
transposed output projections woT[t].T @ ctx[t] -> [D, S]; the host sums
the 8 partials per batch, transposes, and adds bo.

Schedule (single PE stream, ACT-paced):
  - lead-in: k-outer KT(t=0) + Q(t=0) projections (few weight reloads,
    7 PSUM banks held), PE-warmup matmuls cover the input-DMA latency.
  - attention runs per (head-pair, s-half, kv-chunk).  Scores for the
    two heads of a pair are row-tiled (K=64 each, tile_position (0,0) /
    (64,0)) so both stream concurrently through the PE array.  Exp runs
    on the Scalar engine out of PSUM with the mask bias folded into the
    activation bias; the Scalar engine is the pacing engine mid-kernel
    (~92us of exp), so every remaining projection (V, KT t=1, Q t=1,
    out-proj t=0, patmul) is emitted as filler between score groups,
    sharing the scores' two PSUM slots.
  - V gets a ones-column (at a 128-col pitch) so the softmax denominator
    falls out of the context matmul's row 64 for free; denominators are
    reshaped [1,1024]->[128,8] via DMA straight out of PSUM so the
    reciprocal runs on all DVE lanes; a PE outer-product broadcasts the
    reciprocal back across partitions (patmul).
  - out-proj t=1 + remaining groups drain at the end, output DMA per
    128-row chunk overlaps the remaining matmuls.
"""

import contextlib
import sys

if "/opt/trn_rl_repo" not in sys.path:
    sys.path.insert(0, "/opt/trn_rl_repo")

import ml_dtypes
import numpy as np

import concourse.bass as bass  # noqa: F401
import concourse.mybir as mybir
import concourse.tile as tile
from concourse import bacc
from concourse.bass_utils import run_bass_kernel_spmd

B, S, MEM, D, H = 2, 2048, 512, 1024, 16
DH = D // H            # 64
SKV_FULL = MEM + S     # 2560
N_CORES = 8
HPC = 4                # heads per core
F = HPC * DH           # 256 features per core
NK = D // 128          # 8 contraction chunks over D
NT = F // 128          # 2 feature tiles (head pairs) of 128 per core
FP32 = mybir.dt.float32
BF16 = mybir.dt.bfloat16
NEG = -1.0e6


def _build(nj: int):
    """Build the SPMD Bass graph for skv_pad = nj*128 kv positions."""
    skv = nj * 128
    nc = bacc.Bacc("TRN2", target_bir_lowering=False, debug=False,
                   num_devices=N_CORES)

    def din(name, shape, dt=FP32):
        return nc.dram_tensor(name, list(shape), dt, kind="ExternalInput").ap()

    xT = din("xT", [D, S], BF16)      # x[b].T
    cT = din("cT", [D, skv], BF16)    # compacted concat(mems,x)[b].T
    wqT = din("wqT", [D, F], BF16)    # Wq[block].T
    wkT = din("wkT", [D, F], BF16)
    wvT = din("wvT", [D, F], BF16)
    woT = din("woT", [F, D], BF16)    # Wo[:, block].T
    mb = din("mb", [128, nj])         # exp bias: 0 kept, -1e6 padding
    ones64_d = din("ones64", [1, 64], BF16)
    bq2 = din("bq2", [128, NT])       # bq[block] feature-major [p, t]
    bk2 = din("bk2", [128, NT])
    bv2 = din("bv2", [128, NT])
    # two transposed partial outputs: out[t*D:(t+1)*D, :] = woT[t].T @ ctx[t]
    out = nc.dram_tensor("out", [NT * D, S], BF16, kind="ExternalOutput").ap()

    # kv-chunk N-splits for the KT projection (N<=512 per psum bank)
    kt_chunks = []
    off = 0
    while off < skv:
        ln = min(512, skv - off)
        kt_chunks.append((off, ln))
        off += ln
    NKC = len(kt_chunks)

    with tile.TileContext(nc) as tc, contextlib.ExitStack() as big:
        pers = big.enter_context(tc.tile_pool(name="pers", bufs=1))
        pql = big.enter_context(tc.tile_pool(name="pql", bufs=1))
        proj = big.enter_context(tc.tile_pool(name="proj", bufs=1))
        epool = big.enter_context(tc.tile_pool(name="epool", bufs=6))
        ostage = big.enter_context(tc.tile_pool(name="ostage", bufs=8))

        # ---------------- persistent tiles ----------------
        kt = [pers.tile([128, skv], BF16, name=f"kt{t}") for t in range(NT)]
        # per-head Q, zero-padded to the paired kt tile's 128 rows so the
        # scores matmul runs a full-K 128x128 (no tile-mode switches)
        qt = [pers.tile([128, S], BF16, name=f"qt{h}") for h in range(HPC)]
        # vaug layout per (j, h): [V_h (64) | ones | zeros (63)]
        vaug = pers.tile([128, nj * (HPC * 128)], BF16, name="vaug")
        wo_sb = [pers.tile([128, D], BF16, name=f"wo{t}") for t in range(NT)]
        ctxb = [pers.tile([128, S], BF16, name=f"ctxb{t}") for t in range(NT)]
        mb_sb = pers.tile([128, nj], FP32, name="mb_sb")
        bq_sb = pers.tile([128, NT], FP32, name="bq_sb")
        bk_sb = pers.tile([128, NT], FP32, name="bk_sb")
        bv_sb = pers.tile([128, NT], FP32, name="bv_sb")
        ones64_sb = pers.tile([1, 64], BF16, name="ones64_sb")
        dpack = pers.tile([128, HPC * 16], FP32, name="dpack")
        rpack = pers.tile([128, HPC * 16], BF16, name="rpack")
        recip = [pers.tile([1, S], BF16, name=f"recip{h}")
                 for h in range(HPC)]
        dtmp = [pers.tile([1, S], FP32, name=f"dtmp{h}") for h in range(HPC)]
        scr = pers.tile([128, 2], FP32, name="scr")

        xt_sb = [pql.tile([128, S], BF16, name=f"xt{k}") for k in range(NK)]
        wq_sb = [pql.tile([128, F], BF16, name=f"wq{k}") for k in range(NK)]

        ct_sb = [proj.tile([128, skv], BF16, name=f"ct{k}")
                 for k in range(NK)]
        wv_sb = [proj.tile([128, F], BF16, name=f"wv{k}") for k in range(NK)]
        wk_sb = [proj.tile([128, F], BF16, name=f"wk{k}") for k in range(NK)]
        wz = proj.tile([128, 512], BF16, name="wz")

        vview = vaug.rearrange("p (j h e) -> p j h e", j=nj, h=HPC, e=128)

        with contextlib.ExitStack() as attn_scope:
            lead = attn_scope.enter_context(
                tc.tile_pool(name="lead", bufs=1, space="PSUM"))

            # PE warmup: zero-dep matmuls keep the Tensor engine busy while
            # the input DMAs land (HAM clock gate).  wz memset must be the
            # first DVE op so the warmup isn't stuck behind other memsets.
            nc.vector.memset(wz[:], 1.0)
            psw = lead.tile([128, 512], FP32, tag="psw", name="psw")
            for i in range(12):
                nc.tensor.matmul(psw[:], wz[:, :128], wz[:],
                                 start=(i == 0), stop=(i == 11))

            nc.vector.memset(vview[:, :, :, 64:65], 1.0)
            nc.vector.memset(vview[:, :, :, 65:128], 0.0)
            for h in range(HPC):
                zsl = slice(64, 128) if h % 2 == 0 else slice(0, 64)
                nc.vector.memset(qt[h][zsl, :], 0.0)

            # DMA priority order: everything attention needs for (pair 0,
            # s-half 0) first; xt halves 2-3 / wv / wo land mid-attention.
            nc.sync.dma_start(mb_sb[:], mb[:])
            nc.sync.dma_start(bq_sb[:], bq2[:])
            nc.sync.dma_start(bk_sb[:], bk2[:])
            nc.sync.dma_start(bv_sb[:], bv2[:])
            nc.sync.dma_start(ones64_sb[:], ones64_d[:])
            for k in range(NK):
                nc.sync.dma_start(wk_sb[k][:], wkT[k * 128:(k + 1) * 128, :])
            for k in range(NK):
                nc.sync.dma_start(ct_sb[k][:], cT[k * 128:(k + 1) * 128, :])
            for k in range(NK):
                nc.sync.dma_start(wq_sb[k][:], wqT[k * 128:(k + 1) * 128, :])
            for half in range(2):
                hsl = slice(half * 512, (half + 1) * 512)
                for k in range(NK):
                    nc.sync.dma_start(xt_sb[k][:, hsl],
                                      xT[k * 128:(k + 1) * 128, hsl])
            for k in range(NK):
                nc.sync.dma_start(wv_sb[k][:], wvT[k * 128:(k + 1) * 128, :])
            for half in range(2, 4):
                hsl = slice(half * 512, (half + 1) * 512)
                for k in range(NK):
                    nc.sync.dma_start(xt_sb[k][:, hsl],
                                      xT[k * 128:(k + 1) * 128, hsl])
            for t in range(NT):
                nc.sync.dma_start(wo_sb[t][:], woT[t * 128:(t + 1) * 128, :])

            # preload the exp table set (~2.7us) during the lead-in
            nc.scalar.activation(scr[:, 1:2], mb_sb[:, 0:1],
                                 mybir.ActivationFunctionType.Exp)

            def evict_q(t, n, ps):
                nsl = slice(n * 512, (n + 1) * 512)
                nc.vector.tensor_scalar_add(
                    qt[2 * t][0:64, nsl], ps[0:64, :], bq_sb[0:64, t:t + 1])
                nc.vector.tensor_scalar_add(
                    qt[2 * t + 1][64:128, nsl], ps[64:128, :],
                    bq_sb[64:128, t:t + 1])

            # lead-in: KT t=0 + Q t=0 (s-halves 0-1), k-outer so each
            # weight chunk is loaded once
            kps = [lead.tile([128, 512], FP32, tag=f"kps{c}", name=f"kps{c}")
                   for c in range(NKC)]
            qps = [lead.tile([128, 512], FP32, tag=f"qps{n}", name=f"qps{n}")
                   for n in range(2)]
            for k in range(NK):
                for c, (noff, nlen) in enumerate(kt_chunks):
                    nc.tensor.matmul(kps[c][:, :nlen], wk_sb[k][:, 0:128],
                                     ct_sb[k][:, noff:noff + nlen],
                                     start=(k == 0), stop=(k == NK - 1))
                for n in range(2):
                    nc.tensor.matmul(qps[n][:], wq_sb[k][:, 0:128],
                                     xt_sb[k][:, n * 512:(n + 1) * 512],
                                     start=(k == 0), stop=(k == NK - 1))
            for c, (noff, nlen) in enumerate(kt_chunks):
                nc.vector.tensor_scalar_add(
                    kt[0][:, noff:noff + nlen], kps[c][:, :nlen],
                    bk_sb[:, 0:1])
            for n in range(2):
                evict_q(0, n, qps[n])

        # ============ attention (+ woven fillers) ============
        with contextlib.ExitStack() as attn_scope:
            psctx = attn_scope.enter_context(
                tc.tile_pool(name="psctx", bufs=2, space="PSUM"))
            pssc = attn_scope.enter_context(
                tc.tile_pool(name="pssc", bufs=2, space="PSUM"))

            def f_vproj(j):
                def emit():
                    ps = pssc.tile([128, 256], FP32, tag="sc", name=f"vps{j}")
                    for k in range(NK):
                        nc.tensor.matmul(
                            ps[:, :F], ct_sb[k][:, j * 128:(j + 1) * 128],
                            wv_sb[k][:], start=(k == 0), stop=(k == NK - 1))
                    pv = ps[:, :F].rearrange("p (h e) -> p h e", h=HPC)
                    nc.vector.tensor_copy(vview[:, j, :, 0:64], pv)
                return emit

            def f_ktproj(c):
                noff, nlen = kt_chunks[c]

                def emit():
                    ps = pssc.tile([128, 512], FP32, tag="sc",
                                   name=f"k1ps{c}")
                    for k in range(NK):
                        nc.tensor.matmul(ps[:, :nlen], wk_sb[k][:, 128:256],
                                         ct_sb[k][:, noff:noff + nlen],
                                         start=(k == 0), stop=(k == NK - 1))
                    nc.vector.tensor_scalar_add(
                        kt[1][:, noff:noff + nlen], ps[:, :nlen],
                        bk_sb[:, 1:2])
                return emit

            def f_qproj(t, n):
                def emit():
                    ps = pssc.tile([128, 512], FP32, tag="sc",
                                   name=f"q1ps{t}{n}")
                    for k in range(NK):
                        nc.tensor.matmul(
                            ps[:], wq_sb[k][:, t * 128:(t + 1) * 128],
                            xt_sb[k][:, n * 512:(n + 1) * 512],
                            start=(k == 0), stop=(k == NK - 1))
                    evict_q(t, n, ps)
                return emit

            def f_patmul(t, h, sh):
                def emit():
                    rsl = slice((h % 2) * 64, (h % 2) * 64 + 64)
                    for q in range(2):
                        ssl = slice(sh * 1024 + q * 512,
                                    sh * 1024 + (q + 1) * 512)
                        ps = pssc.tile([128, 512], FP32, tag="sc",
                                       name=f"pat{h}{sh}{q}")
                        nc.tensor.matmul(ps[0:64, :], ones64_sb[:],
                                         recip[h][:, ssl],
                                         start=True, stop=True)
                        nc.vector.tensor_mul(ctxb[t][rsl, ssl],
                                             ctxb[t][rsl, ssl], ps[0:64, :])
                return emit

            def f_oproj(t, d):
                def emit():
                    dsl = slice(d * 128, (d + 1) * 128)
                    for q in range(4):
                        ssl = slice(q * 512, (q + 1) * 512)
                        ps = pssc.tile([128, 512], FP32, tag="sc",
                                       name=f"ops{t}{d}{q}")
                        nc.tensor.matmul(ps[:], wo_sb[t][:, dsl],
                                         ctxb[t][:, ssl],
                                         start=True, stop=True)
                        ob = ostage.tile([128, 512], BF16, tag="ob",
                                         name=f"ob{t}{d}{q}")
                        nc.vector.tensor_copy(ob[:], ps[:])
                        nc.sync.dma_start(
                            out[t * D + d * 128:t * D + (d + 1) * 128, ssl],
                            ob[:])
                return emit

            def f_dummy():
                # PE keep-warm: matmuls into a dead psum slot
                ps = pssc.tile([128, 512], FP32, tag="sc", name="dps")
                for i in range(2):
                    nc.tensor.matmul(ps[:], wz[:, :128], wz[:],
                                     start=(i == 0), stop=(i == 1))

            # filler schedule: lists per (pair, s-half) indexed [t][sh];
            # each entry is a list of closures for one j-iteration.
            def sched(t, sh, j):
                if t == 0 and sh == 0:
                    out_ = []
                    if j + 1 < nj:
                        out_.append(f_vproj(j + 1))
                    if j == 1:
                        out_.append(f_qproj(0, 2))
                    elif j == 2:
                        out_.append(f_qproj(0, 3))
                    elif j == nj - 1:
                        out_.append(f_ktproj(0))
                    return out_
                if t == 0 and sh == 1:
                    m = {0: [f_patmul(0, 0, 0), f_patmul(0, 1, 0)]}
                    for c in range(1, NKC):
                        m[c] = [f_ktproj(c)]
                    for n in range(4):
                        m[NKC + n] = [f_qproj(1, n)]
                    return m.get(j, [None])
                if t == 1 and sh == 0:
                    m = {}
                    for d in range(4):
                        m[1 + d] = [f_oproj(0, d)]
                    return m.get(j, [None])
                m = {0: [f_patmul(1, 2, 0), f_patmul(1, 3, 0)]}
                for d in range(4):
                    m[1 + d] = [f_oproj(0, 4 + d)]
                return m.get(j, [None])

            # V chunk 0 must exist before the first ctx matmul
            f_vproj(0)()

            for t in range(NT):
                for sh in range(2):
                    shsl = slice(sh * 1024, (sh + 1) * 1024)
                    cps = [psctx.tile([128, 1024], FP32, tag="ctx",
                                      name=f"cps{t}{sh}{r}")
                           for r in range(2)]
                    etiles = [[None] * nj for _ in range(2)]

                    def emit_ctx(j, t=t, cps=cps, etiles=etiles):
                        for r in range(2):
                            h = 2 * t + r
                            vsl = vaug[:, j * (HPC * 128) + h * 128:
                                       j * (HPC * 128) + h * 128 + 65]
                            for q in range(2):
                                nc.tensor.matmul(
                                    cps[r][0:65, q * 512:(q + 1) * 512],
                                    vsl,
                                    etiles[r][j][:, q * 512:(q + 1) * 512],
                                    start=(j == 0), stop=(j == nj - 1))

                    for j in range(nj):
                        for r in range(2):
                            h = 2 * t + r
                            ps = pssc.tile([128, 1024], FP32, tag="sc",
                                           name=f"sps{t}{sh}{j}{r}")
                            for q in range(2):
                                nc.tensor.matmul(
                                    ps[:, q * 512:(q + 1) * 512],
                                    kt[t][:, j * 128:(j + 1) * 128],
                                    qt[h][:, sh * 1024 + q * 512:
                                           sh * 1024 + (q + 1) * 512],
                                    start=True, stop=True)
                            e = epool.tile([128, 1024], BF16, tag="expT",
                                           name=f"e{t}{sh}{j}{r}")
                            etiles[r][j] = e
                            nc.scalar.activation(
                                e[:], ps[:],
                                mybir.ActivationFunctionType.Exp,
                                bias=mb_sb[:, j:j + 1], scale=1.0 / 8.0)
                        # ctx for the previous chunk (its exp is long done)
                        if j > 0:
                            emit_ctx(j - 1)
                        for f in sched(t, sh, j):
                            if f is None:
                                f_dummy()
                            else:
                                f()
                    emit_ctx(nj - 1)
                    # evict pair ctx + denominators for this s-half
                    for r in range(2):
                        h = 2 * t + r
                        rsl = slice(r * 64, (r + 1) * 64)
                        nc.vector.tensor_copy(ctxb[t][rsl, shsl],
                                              cps[r][0:64, :])
                        dsl = slice(h * 16 + sh * 8, h * 16 + sh * 8 + 8)
                        nc.vector.tensor_copy(dtmp[h][:, shsl],
                                              cps[r][64:65, :])
                        nc.sync.dma_start(dpack[:, dsl], dtmp[h][:, shsl])
                        with nc.allow_low_precision(
                                reason="bf16 recip feeds bf16 broadcast mm"):
                            nc.vector.reciprocal(rpack[:, dsl],
                                                 dpack[:, dsl])
                        nc.sync.dma_start(recip[h][:, shsl], rpack[:, dsl])
                # pair done: normalize the last s-half + bv, ready for oproj
                f_patmul(t, 2 * t, 1)()
                f_patmul(t, 2 * t + 1, 1)()
                nc.vector.tensor_scalar_add(ctxb[t][:], ctxb[t][:],
                                            bv_sb[:, t:t + 1])

        # ============ tail: out-proj t=1 on a wide psum pool ============
        with contextlib.ExitStack() as tail_scope:
            tailps = tail_scope.enter_context(
                tc.tile_pool(name="tailps", bufs=6, space="PSUM"))
            for d in range(NK):
                dsl = slice(d * 128, (d + 1) * 128)
                for q in range(4):
                    ssl = slice(q * 512, (q + 1) * 512)
                    ps = tailps.tile([128, 512], FP32, tag="tl",
                                     name=f"tps{d}{q}")
                    nc.tensor.matmul(ps[:], wo_sb[1][:, dsl],
                                     ctxb[1][:, ssl], start=True, stop=True)
                    ob = ostage.tile([128, 512], BF16, tag="ob",
                                     name=f"tob{d}{q}")
                    if q % 2 == 1:
                        nc.scalar.copy(ob[:], ps[:])
                    else:
                        nc.vector.tensor_copy(ob[:], ps[:])
                    nc.sync.dma_start(out[D + d * 128:D + (d + 1) * 128,
                                          ssl], ob[:])

    nc.compile()
    return nc


_CACHE = {}


def _graph(nj):
    if nj not in _CACHE:
        _CACHE[nj] = _build(nj)
    return _CACHE[nj]


def _prep_inputs(x, mems, mask, Wq, bq, Wk, bk, Wv, bv, Wo, bo):
    """Shard + preprocess on host. Returns (in_maps, nj)."""
    c = np.concatenate([mems, x], axis=1)          # [B, SKV_FULL, D]
    keep = [np.nonzero(mask[b] != 0)[0] for b in range(B)]
    n_eff = [len(k) for k in keep]
    nj = max(1, (max(n_eff) + 127) // 128)
    skv = nj * 128

    per_batch = []
    for b in range(B):
        ne = n_eff[b]
        cTb = np.zeros((D, skv), ml_dtypes.bfloat16)
        cTb[:, :ne] = c[b][keep[b]].T.astype(ml_dtypes.bfloat16)
        xTb = np.ascontiguousarray(x[b].T.astype(ml_dtypes.bfloat16))
        mbb = np.full(skv, NEG, np.float32)
        mbb[:ne] = 0.0
        mbb = np.ascontiguousarray(mbb.reshape(nj, 128).T)   # [128, nj]
        per_batch.append((xTb, cTb, mbb))

    def fmaj(v):   # [F] -> [128, NT] feature-major
        return np.ascontiguousarray(v.reshape(NT, 128).T.astype(np.float32))

    in_maps = []
    for core in range(N_CORES):
        b, hb = divmod(core, HPC)
        fs = slice(hb * F, (hb + 1) * F)
        xTb, cTb, mbb = per_batch[b]
        in_maps.append({
            "xT": xTb,
            "cT": cTb,
            "wqT": np.ascontiguousarray(Wq[fs, :].T.astype(ml_dtypes.bfloat16)),
            "wkT": np.ascontiguousarray(Wk[fs, :].T.astype(ml_dtypes.bfloat16)),
            "wvT": np.ascontiguousarray(Wv[fs, :].T.astype(ml_dtypes.bfloat16)),
            "woT": np.ascontiguousarray(Wo[:, fs].T.astype(ml_dtypes.bfloat16)),
            "mb": mbb,
            "ones64": np.ones((1, 64), ml_dtypes.bfloat16),
            "bq2": fmaj(bq[fs]),
            "bk2": fmaj(bk[fs]),
            "bv2": fmaj(bv[fs]),
        })
    return in_maps, nj


def _register_ntff_hook():
    try:
        from antenv.axon_hooks import (get_axon_ntff_profile_hook,
                                       set_axon_ntff_profile_hook)
    except ImportError:
        import types

        import antenv
        m = types.ModuleType("antenv.axon_hooks")
        m._hook = None
        m.set_axon_ntff_profile_hook = lambda h: setattr(m, "_hook", h)
        m.get_axon_ntff_profile_hook = lambda: m._hook
        sys.modules["antenv.axon_hooks"] = m
        antenv.axon_hooks = m
        get_axon_ntff_profile_hook = m.get_axon_ntff_profile_hook
        set_axon_ntff_profile_hook = m.set_axon_ntff_profile_hook
    if get_axon_ntff_profile_hook() is None:
        from trn_agent_boot.trn_boot import _ntff_profile_via_ctypes
        set_axon_ntff_profile_hook(
            _ntff_profile_via_ctypes("/opt/axon/libaxon_pjrt.so"))


def _run(inputs, trace=False, trace_kwargs=None):
    x = np.asarray(inputs["x"], np.float32)
    mems = np.asarray(inputs["mems"], np.float32)
    mask = np.asarray(inputs["mask"])
    Wq = np.asarray(inputs["Wq"], np.float32)
    bq = np.asarray(inputs["bq"], np.float32)
    Wk = np.asarray(inputs["Wk"], np.float32)
    bk = np.asarray(inputs["bk"], np.float32)
    Wv = np.asarray(inputs["Wv"], np.float32)
    bv = np.asarray(inputs["bv"], np.float32)
    Wo = np.asarray(inputs["Wo"], np.float32)
    bo = np.asarray(inputs["bo"], np.float32)

    in_maps, nj = _prep_inputs(x, mems, mask, Wq, bq, Wk, bk, Wv, bv, Wo, bo)
    nc = _graph(nj)

    if trace:
        _register_ntff_hook()

    res = run_bass_kernel_spmd(nc, in_maps, core_ids=list(range(N_CORES)),
                               trace=trace, **(trace_kwargs or {}))

    out = np.empty((B, S, D), np.float32)
    for b in range(B):
        acc = None
        for hb in range(HPC):
            o = res.results[b * HPC + hb]["out"].astype(np.float32)
            part = o[:D] + o[D:]
            acc = part if acc is None else acc + part
        out[b] = acc.T + bo[None, :]
    return out, res


def kernel(**inputs) -> np.ndarray:
    out, _ = _run(inputs, trace=False)
    return out


# revision 16
# speedup vs baseline: 1.2335x; 1.0738x over previous
"""Trainium2 Bass kernel for nn_MHA_28922309771622.

Multi-head attention with memory prefix (mems prepended to K/V), boolean
mask over KV positions, 16 heads, D=1024, S=2048, MEM=512, fp32.

Sharding: 8 cores = 2 (batch) x 4 (head blocks of 4 heads).  Each core
computes its head block's Q/K/V projections, attention, and two partial
transposed output projections woT[t].T @ ctx[t] -> [D, S]; the host sums
the 8 partials per batch, transposes, and adds bo.

Schedule (single PE stream, ACT-paced):
  - lead-in: k-outer KT(t=0) + Q(t=0) projections (few weight reloads,
    7 PSUM banks held), PE-warmup matmuls cover the input-DMA latency.
  - attention runs per (head-pair, s-half, kv-chunk).  Scores for the
    two heads of a pair are row-tiled (K=64 each, tile_position (0,0) /
    (64,0)) so both stream concurrently through the PE array.  Exp runs
    on the Scalar engine out of PSUM with the mask bias folded into the
    activation bias; the Scalar engine is the pacing engine mid-kernel
    (~92us of exp), so every remaining projection (V, KT t=1, Q t=1,
    out-proj t=0, patmul) is emitted as filler between score groups,
    sharing the scores' two PSUM slots.
  - V gets a ones-column (at a 128-col pitch) so the softmax denominator
    falls out of the context matmul's row 64 for free; denominators are
    reshaped [1,1024]->[128,8] via DMA straight out of PSUM so the
    reciprocal runs on all DVE lanes; a PE outer-product broadcasts the
    reciprocal back across partitions (patmul).
  - out-proj t=1 + remaining groups drain at the end, output DMA per
    128-row chunk overlaps the remaining matmuls.
"""

import contextlib
import sys

if "/opt/trn_rl_repo" not in sys.path:
    sys.path.insert(0, "/opt/trn_rl_repo")

import ml_dtypes
import numpy as np

import concourse.bass as bass  # noqa: F401
import concourse.mybir as mybir
import concourse.tile as tile
from concourse import bacc
from concourse.bass_utils import run_bass_kernel_spmd

B, S, MEM, D, H = 2, 2048, 512, 1024, 16
DH = D // H            # 64
SKV_FULL = MEM + S     # 2560
N_CORES = 8
HPC = 4                # heads per core
F = HPC * DH           # 256 features per core
NK = D // 128          # 8 contraction chunks over D
NT = F // 128          # 2 feature tiles (head pairs) of 128 per core
FP32 = mybir.dt.float32
BF16 = mybir.dt.bfloat16
NEG = -1.0e6


def _build(nj: int):
    """Build the SPMD Bass graph for skv_pad = nj*128 kv positions."""
    skv = nj * 128
    nc = bacc.Bacc("TRN2", target_bir_lowering=False, debug=False,
                   num_devices=N_CORES)

    def din(name, shape, dt=FP32):
        return nc.dram_tensor(name, list(shape), dt, kind="ExternalInput").ap()

    xT = din("xT", [D, S], BF16)      # x[b].T
    cT = din("cT", [D, skv], BF16)    # compacted concat(mems,x)[b].T
    wqT = din("wqT", [D, F], BF16)    # Wq[block].T
    wkT = din("wkT", [D, F], BF16)
    wvT = din("wvT", [D, F], BF16)
    woT = din("woT", [F, D], BF16)    # Wo[:, block].T
    mb = din("mb", [128, nj])         # exp bias: 0 kept, -1e6 padding
    ones64_d = din("ones64", [1, 64], BF16)
    bq2 = din("bq2", [128, NT])       # bq[block] feature-major [p, t]
    bk2 = din("bk2", [128, NT])
    bv2 = din("bv2", [128, NT])
    # two transposed partial outputs: out[t*D:(t+1)*D, :] = woT[t].T @ ctx[t]
    out = nc.dram_tensor("out", [NT * D, S], BF16, kind="ExternalOutput").ap()

    # kv-chunk N-splits for the KT projection (N<=512 per psum bank)
    kt_chunks = []
    off = 0
    while off < skv:
        ln = min(512, skv - off)
        kt_chunks.append((off, ln))
        off += ln
    NKC = len(kt_chunks)

    with tile.TileContext(nc) as tc, contextlib.ExitStack() as big:
        pers = big.enter_context(tc.tile_pool(name="pers", bufs=1))
        pql = big.enter_context(tc.tile_pool(name="pql", bufs=1))
        proj = big.enter_context(tc.tile_pool(name="proj", bufs=1))
        epool = big.enter_context(tc.tile_pool(name="epool", bufs=6))
        ostage = big.enter_context(tc.tile_pool(name="ostage", bufs=8))

        # ---------------- persistent tiles ----------------
        kt = [pers.tile([128, skv], BF16, name=f"kt{t}") for t in range(NT)]
        # per-head Q, zero-padded to the paired kt tile's 128 rows so the
        # scores matmul runs a full-K 128x128 (no tile-mode switches)
        qt = [pers.tile([128, S], BF16, name=f"qt{h}") for h in range(HPC)]
        # vaug layout per (j, h): [V_h (64) | ones | zeros (63)]
        vaug = pers.tile([128, nj * (HPC * 128)], BF16, name="vaug")
        wo_sb = [pers.tile([128, D], BF16, name=f"wo{t}") for t in range(NT)]
        ctxb = [pers.tile([128, S], BF16, name=f"ctxb{t}") for t in range(NT)]
        mb_sb = pers.tile([128, nj], FP32, name="mb_sb")
        bq_sb = pers.tile([128, NT], FP32, name="bq_sb")
        bk_sb = pers.tile([128, NT], FP32, name="bk_sb")
        bv_sb = pers.tile([128, NT], FP32, name="bv_sb")
        ones64_sb = pers.tile([1, 64], BF16, name="ones64_sb")
        dpack = pers.tile([128, HPC * 16], FP32, name="dpack")
        rpack = pers.tile([128, HPC * 16], BF16, name="rpack")
        recip = [pers.tile([1, S], BF16, name=f"recip{h}")
                 for h in range(HPC)]
        dtmp = [pers.tile([1, S], FP32, name=f"dtmp{h}") for h in range(HPC)]
        scr = pers.tile([128, 2], FP32, name="scr")

        xt_sb = [pql.tile([128, S], BF16, name=f"xt{k}") for k in range(NK)]
        wq_sb = [pql.tile([128, F], BF16, name=f"wq{k}") for k in range(NK)]

        ct_sb = [proj.tile([128, skv], BF16, name=f"ct{k}")
                 for k in range(NK)]
        wv_sb = [proj.tile([128, F], BF16, name=f"wv{k}") for k in range(NK)]
        wk_sb = [proj.tile([128, F], BF16, name=f"wk{k}") for k in range(NK)]
        wz = proj.tile([128, 512], BF16, name="wz")

        vview = vaug.rearrange("p (j h e) -> p j h e", j=nj, h=HPC, e=128)

        with contextlib.ExitStack() as attn_scope:
            lead = attn_scope.enter_context(
                tc.tile_pool(name="lead", bufs=1, space="PSUM"))

            # PE warmup: zero-dep matmuls keep the Tensor engine busy while
            # the input DMAs land (HAM clock gate).  wz memset must be the
            # first DVE op so the warmup isn't stuck behind other memsets.
            nc.vector.memset(wz[:], 1.0)
            psw = lead.tile([128, 512], FP32, tag="psw", name="psw")
            for i in range(12):
                nc.tensor.matmul(psw[:], wz[:, :128], wz[:],
                                 start=(i == 0), stop=(i == 11))

            nc.vector.memset(vview[:, :, :, 64:65], 1.0)
            nc.vector.memset(vview[:, :, :, 65:128], 0.0)
            for h in range(HPC):
                zsl = slice(64, 128) if h % 2 == 0 else slice(0, 64)
                nc.vector.memset(qt[h][zsl, :], 0.0)

            # DMA priority order: everything attention needs for (pair 0,
            # s-half 0) first; xt halves 2-3 / wv / wo land mid-attention.
            nc.sync.dma_start(mb_sb[:], mb[:])
            nc.sync.dma_start(bq_sb[:], bq2[:])
            nc.sync.dma_start(bk_sb[:], bk2[:])
            nc.sync.dma_start(bv_sb[:], bv2[:])
            nc.sync.dma_start(ones64_sb[:], ones64_d[:])
            for k in range(NK):
                nc.sync.dma_start(wk_sb[k][:], wkT[k * 128:(k + 1) * 128, :])
            for k in range(NK):
                nc.sync.dma_start(ct_sb[k][:], cT[k * 128:(k + 1) * 128, :])
            for k in range(NK):
                nc.sync.dma_start(wq_sb[k][:], wqT[k * 128:(k + 1) * 128, :])
            for k in range(NK):
                nc.sync.dma_start(xt_sb[k][:, 0:1024],
                                  xT[k * 128:(k + 1) * 128, 0:1024])
            for k in range(NK):
                nc.sync.dma_start(wv_sb[k][:], wvT[k * 128:(k + 1) * 128, :])
            for k in range(NK):
                nc.sync.dma_start(xt_sb[k][:, 1024:2048],
                                  xT[k * 128:(k + 1) * 128, 1024:2048])
            for t in range(NT):
                nc.sync.dma_start(wo_sb[t][:], woT[t * 128:(t + 1) * 128, :])

            # preload the exp table set (~2.7us) during the lead-in
            nc.scalar.activation(scr[:, 1:2], mb_sb[:, 0:1],
                                 mybir.ActivationFunctionType.Exp)

            def evict_q(t, n, ps):
                nsl = slice(n * 512, (n + 1) * 512)
                nc.vector.tensor_scalar_add(
                    qt[2 * t][0:64, nsl], ps[0:64, :], bq_sb[0:64, t:t + 1])
                nc.vector.tensor_scalar_add(
                    qt[2 * t + 1][64:128, nsl], ps[64:128, :],
                    bq_sb[64:128, t:t + 1])

            # lead-in: KT t=0 + Q t=0 (s-halves 0-1), k-outer so each
            # weight chunk is loaded once
            kps = [lead.tile([128, 512], FP32, tag=f"kps{c}", name=f"kps{c}")
                   for c in range(NKC)]
            qps = [lead.tile([128, 512], FP32, tag=f"qps{n}", name=f"qps{n}")
                   for n in range(2)]
            for k in range(NK):
                for c, (noff, nlen) in enumerate(kt_chunks):
                    nc.tensor.matmul(kps[c][:, :nlen], wk_sb[k][:, 0:128],
                                     ct_sb[k][:, noff:noff + nlen],
                                     start=(k == 0), stop=(k == NK - 1))
                for n in range(2):
                    nc.tensor.matmul(qps[n][:], wq_sb[k][:, 0:128],
                                     xt_sb[k][:, n * 512:(n + 1) * 512],
                                     start=(k == 0), stop=(k == NK - 1))
            for c, (noff, nlen) in enumerate(kt_chunks):
                nc.vector.tensor_scalar_add(
                    kt[0][:, noff:noff + nlen], kps[c][:, :nlen],
                    bk_sb[:, 0:1])
            for n in range(2):
                evict_q(0, n, qps[n])

        # ============ attention (+ woven fillers) ============
        with contextlib.ExitStack() as attn_scope:
            psctx = attn_scope.enter_context(
                tc.tile_pool(name="psctx", bufs=2, space="PSUM"))
            pssc = attn_scope.enter_context(
                tc.tile_pool(name="pssc", bufs=2, space="PSUM"))

            def f_vproj(j):
                def emit():
                    ps = pssc.tile([128, 256], FP32, tag="sc", name=f"vps{j}")
                    for k in range(NK):
                        nc.tensor.matmul(
                            ps[:, :F], ct_sb[k][:, j * 128:(j + 1) * 128],
                            wv_sb[k][:], start=(k == 0), stop=(k == NK - 1))
                    pv = ps[:, :F].rearrange("p (h e) -> p h e", h=HPC)
                    nc.vector.tensor_copy(vview[:, j, :, 0:64], pv)
                return emit

            def f_ktproj(c):
                noff, nlen = kt_chunks[c]

                def emit():
                    ps = pssc.tile([128, 512], FP32, tag="sc",
                                   name=f"k1ps{c}")
                    for k in range(NK):
                        nc.tensor.matmul(ps[:, :nlen], wk_sb[k][:, 128:256],
                                         ct_sb[k][:, noff:noff + nlen],
                                         start=(k == 0), stop=(k == NK - 1))
                    nc.vector.tensor_scalar_add(
                        kt[1][:, noff:noff + nlen], ps[:, :nlen],
                        bk_sb[:, 1:2])
                return emit

            def f_qproj(t, n):
                def emit():
                    ps = pssc.tile([128, 512], FP32, tag="sc",
                                   name=f"q1ps{t}{n}")
                    for k in range(NK):
                        nc.tensor.matmul(
                            ps[:], wq_sb[k][:, t * 128:(t + 1) * 128],
                            xt_sb[k][:, n * 512:(n + 1) * 512],
                            start=(k == 0), stop=(k == NK - 1))
                    evict_q(t, n, ps)
                return emit

            def f_patmul(t, h, sh):
                def emit():
                    rsl = slice((h % 2) * 64, (h % 2) * 64 + 64)
                    for q in range(2):
                        ssl = slice(sh * 1024 + q * 512,
                                    sh * 1024 + (q + 1) * 512)
                        ps = pssc.tile([128, 512], FP32, tag="sc",
                                       name=f"pat{h}{sh}{q}")
                        nc.tensor.matmul(ps[0:64, :], ones64_sb[:],
                                         recip[h][:, ssl],
                                         start=True, stop=True)
                        nc.vector.tensor_mul(ctxb[t][rsl, ssl],
                                             ctxb[t][rsl, ssl], ps[0:64, :])
                return emit

            def f_oproj(t, d):
                def emit():
                    dsl = slice(d * 128, (d + 1) * 128)
                    for q in range(4):
                        ssl = slice(q * 512, (q + 1) * 512)
                        ps = pssc.tile([128, 512], FP32, tag="sc",
                                       name=f"ops{t}{d}{q}")
                        nc.tensor.matmul(ps[:], wo_sb[t][:, dsl],
                                         ctxb[t][:, ssl],
                                         start=True, stop=True)
                        ob = ostage.tile([128, 512], BF16, tag="ob",
                                         name=f"ob{t}{d}{q}")
                        nc.vector.tensor_copy(ob[:], ps[:])
                        nc.sync.dma_start(
                            out[t * D + d * 128:t * D + (d + 1) * 128, ssl],
                            ob[:])
                return emit

            def f_dummy():
                # PE keep-warm: matmuls into a dead psum slot
                ps = pssc.tile([128, 512], FP32, tag="sc", name="dps")
                for i in range(2):
                    nc.tensor.matmul(ps[:], wz[:, :128], wz[:],
                                     start=(i == 0), stop=(i == 1))

            def f_bv(t):
                def emit():
                    nc.vector.tensor_scalar_add(ctxb[t][:], ctxb[t][:],
                                                bv_sb[:, t:t + 1])
                return emit

            # filler schedule per (pair, s-half): {j: [closures]}.  patmul
            # fillers sit >=2 iterations past their denominators' eviction
            # so their matmuls never block the in-order PE queue on the
            # reciprocal DMA chain.  Entries beyond nj-1 are flushed at the
            # end of the section (correctness with small nj).
            def sched(t, sh):
                if t == 0 and sh == 0:
                    m = {j: [f_vproj(j + 1)] for j in range(nj - 1)}
                    m.setdefault(1, []).append(f_qproj(0, 2))
                    m.setdefault(2, []).append(f_qproj(0, 3))
                    m.setdefault(nj - 1, []).append(f_ktproj(0))
                    return m
                if t == 0 and sh == 1:
                    m = {0: [f_ktproj(1)], 1: [f_ktproj(2)],
                         2: [f_patmul(0, 0, 0), f_qproj(1, 0)],
                         3: [f_patmul(0, 1, 0), f_qproj(1, 1)],
                         4: [f_qproj(1, 2)], 5: [f_qproj(1, 3)]}
                    return m
                if t == 1 and sh == 0:
                    m = {2: [f_patmul(0, 0, 1)],
                         3: [f_patmul(0, 1, 1), f_bv(0)]}
                    for d in range(4):
                        m[4 + d] = [f_oproj(0, d)]
                    return m
                m = {2: [f_patmul(1, 2, 0)], 3: [f_patmul(1, 3, 0)]}
                for d in range(4):
                    m[4 + d] = [f_oproj(0, 4 + d)]
                return m

            # V chunk 0 must exist before the first ctx matmul
            f_vproj(0)()

            for t in range(NT):
                for sh in range(2):
                    shsl = slice(sh * 1024, (sh + 1) * 1024)
                    plan = sched(t, sh)
                    cps = [psctx.tile([128, 1024], FP32, tag="ctx",
                                      name=f"cps{t}{sh}{r}")
                           for r in range(2)]
                    etiles = [[None] * nj for _ in range(2)]

                    def emit_ctx(j, t=t, cps=cps, etiles=etiles):
                        for r in range(2):
                            h = 2 * t + r
                            vsl = vaug[:, j * (HPC * 128) + h * 128:
                                       j * (HPC * 128) + h * 128 + 65]
                            for q in range(2):
                                nc.tensor.matmul(
                                    cps[r][0:65, q * 512:(q + 1) * 512],
                                    vsl,
                                    etiles[r][j][:, q * 512:(q + 1) * 512],
                                    start=(j == 0), stop=(j == nj - 1))

                    for j in range(nj):
                        for r in range(2):
                            h = 2 * t + r
                            ps = pssc.tile([128, 1024], FP32, tag="sc",
                                           name=f"sps{t}{sh}{j}{r}")
                            for q in range(2):
                                nc.tensor.matmul(
                                    ps[:, q * 512:(q + 1) * 512],
                                    kt[t][:, j * 128:(j + 1) * 128],
                                    qt[h][:, sh * 1024 + q * 512:
                                           sh * 1024 + (q + 1) * 512],
                                    start=True, stop=True)
                            e = epool.tile([128, 1024], BF16, tag="expT",
                                           name=f"e{t}{sh}{j}{r}")
                            etiles[r][j] = e
                            nc.scalar.activation(
                                e[:], ps[:],
                                mybir.ActivationFunctionType.Exp,
                                bias=mb_sb[:, j:j + 1], scale=1.0 / 8.0)
                        # ctx for the previous chunk (its exp is long done)
                        if j > 0:
                            emit_ctx(j - 1)
                        fl = plan.pop(j, None)
                        if fl is None:
                            f_dummy()
                        else:
                            for f in fl:
                                f()
                    for jj in sorted(plan):
                        for f in plan.pop(jj):
                            f()
                    emit_ctx(nj - 1)
                    # evict pair ctx + denominators for this s-half
                    for r in range(2):
                        h = 2 * t + r
                        rsl = slice(r * 64, (r + 1) * 64)
                        if r == 0:
                            nc.vector.tensor_copy(ctxb[t][rsl, shsl],
                                                  cps[r][0:64, :])
                        else:
                            nc.scalar.copy(ctxb[t][rsl, shsl],
                                           cps[r][0:64, :])
                        dsl = slice(h * 16 + sh * 8, h * 16 + sh * 8 + 8)
                        nc.scalar.copy(dtmp[h][:, shsl], cps[r][64:65, :])
                        nc.sync.dma_start(dpack[:, dsl], dtmp[h][:, shsl])
                        with nc.allow_low_precision(
                                reason="bf16 recip feeds bf16 broadcast mm"):
                            nc.vector.reciprocal(rpack[:, dsl],
                                                 dpack[:, dsl])
                        nc.sync.dma_start(recip[h][:, shsl], rpack[:, dsl])
                if t == 1:
                    # last pair: normalize + bv inline, feeding the tail
                    f_patmul(1, 2, 1)()
                    f_patmul(1, 3, 1)()
                    f_bv(1)()

        # ============ tail: out-proj t=1 on a wide psum pool ============
        with contextlib.ExitStack() as tail_scope:
            tailps = tail_scope.enter_context(
                tc.tile_pool(name="tailps", bufs=6, space="PSUM"))
            for d in range(NK):
                dsl = slice(d * 128, (d + 1) * 128)
                for q in range(4):
                    ssl = slice(q * 512, (q + 1) * 512)
                    ps = tailps.tile([128, 512], FP32, tag="tl",
                                     name=f"tps{d}{q}")
                    nc.tensor.matmul(ps[:], wo_sb[1][:, dsl],
                                     ctxb[1][:, ssl], start=True, stop=True)
                    ob = ostage.tile([128, 512], BF16, tag="ob",
                                     name=f"tob{d}{q}")
                    if q % 2 == 1:
                        nc.scalar.copy(ob[:], ps[:])
                    else:
                        nc.vector.tensor_copy(ob[:], ps[:])
                    nc.sync.dma_start(out[D + d * 128:D + (d + 1) * 128,
                                          ssl], ob[:])

    nc.compile()
    return nc


_CACHE = {}


def _graph(nj):
    if nj not in _CACHE:
        _CACHE[nj] = _build(nj)
    return _CACHE[nj]


def _prep_inputs(x, mems, mask, Wq, bq, Wk, bk, Wv, bv, Wo, bo):
    """Shard + preprocess on host. Returns (in_maps, nj)."""
    c = np.concatenate([mems, x], axis=1)          # [B, SKV_FULL, D]
    keep = [np.nonzero(mask[b] != 0)[0] for b in range(B)]
    n_eff = [len(k) for k in keep]
    nj = max(1, (max(n_eff) + 127) // 128)
    skv = nj * 128

    per_batch = []
    for b in range(B):
        ne = n_eff[b]
        cTb = np.zeros((D, skv), ml_dtypes.bfloat16)
        cTb[:, :ne] = c[b][keep[b]].T.astype(ml_dtypes.bfloat16)
        xTb = np.ascontiguousarray(x[b].T.astype(ml_dtypes.bfloat16))
        mbb = np.full(skv, NEG, np.float32)
        mbb[:ne] = 0.0
        mbb = np.ascontiguousarray(mbb.reshape(nj, 128).T)   # [128, nj]
        per_batch.append((xTb, cTb, mbb))

    def fmaj(v):   # [F] -> [128, NT] feature-major
        return np.ascontiguousarray(v.reshape(NT, 128).T.astype(np.float32))

    in_maps = []
    for core in range(N_CORES):
        b, hb = divmod(core, HPC)
        fs = slice(hb * F, (hb + 1) * F)
        xTb, cTb, mbb = per_batch[b]
        in_maps.append({
            "xT": xTb,
            "cT": cTb,
            "wqT": np.ascontiguousarray(Wq[fs, :].T.astype(ml_dtypes.bfloat16)),
            "wkT": np.ascontiguousarray(Wk[fs, :].T.astype(ml_dtypes.bfloat16)),
            "wvT": np.ascontiguousarray(Wv[fs, :].T.astype(ml_dtypes.bfloat16)),
            "woT": np.ascontiguousarray(Wo[:, fs].T.astype(ml_dtypes.bfloat16)),
            "mb": mbb,
            "ones64": np.ones((1, 64), ml_dtypes.bfloat16),
            "bq2": fmaj(bq[fs]),
            "bk2": fmaj(bk[fs]),
            "bv2": fmaj(bv[fs]),
        })
    return in_maps, nj


def _register_ntff_hook():
    try:
        from antenv.axon_hooks import (get_axon_ntff_profile_hook,
                                       set_axon_ntff_profile_hook)
    except ImportError:
        import types

        import antenv
        m = types.ModuleType("antenv.axon_hooks")
        m._hook = None
        m.set_axon_ntff_profile_hook = lambda h: setattr(m, "_hook", h)
        m.get_axon_ntff_profile_hook = lambda: m._hook
        sys.modules["antenv.axon_hooks"] = m
        antenv.axon_hooks = m
        get_axon_ntff_profile_hook = m.get_axon_ntff_profile_hook
        set_axon_ntff_profile_hook = m.set_axon_ntff_profile_hook
    if get_axon_ntff_profile_hook() is None:
        from trn_agent_boot.trn_boot import _ntff_profile_via_ctypes
        set_axon_ntff_profile_hook(
            _ntff_profile_via_ctypes("/opt/axon/libaxon_pjrt.so"))


def _run(inputs, trace=False, trace_kwargs=None):
    x = np.asarray(inputs["x"], np.float32)
    mems = np.asarray(inputs["mems"], np.float32)
    mask = np.asarray(inputs["mask"])
    Wq = np.asarray(inputs["Wq"], np.float32)
    bq = np.asarray(inputs["bq"], np.float32)
    Wk = np.asarray(inputs["Wk"], np.float32)
    bk = np.asarray(inputs["bk"], np.float32)
    Wv = np.asarray(inputs["Wv"], np.float32)
    bv = np.asarray(inputs["bv"], np.float32)
    Wo = np.asarray(inputs["Wo"], np.float32)
    bo = np.asarray(inputs["bo"], np.float32)

    in_maps, nj = _prep_inputs(x, mems, mask, Wq, bq, Wk, bk, Wv, bv, Wo, bo)
    nc = _graph(nj)

    if trace:
        _register_ntff_hook()

    res = run_bass_kernel_spmd(nc, in_maps, core_ids=list(range(N_CORES)),
                               trace=trace, **(trace_kwargs or {}))

    out = np.empty((B, S, D), np.float32)
    for b in range(B):
        acc = None
        for hb in range(HPC):
            o = res.results[b * HPC + hb]["out"].astype(np.float32)
            part = o[:D] + o[D:]
            acc = part if acc is None else acc + part
        out[b] = acc.T + bo[None, :]
    return out, res


def kernel(**inputs) -> np.ndarray:
    out, _ = _run(inputs, trace=False)
    return out
